# revision 1
# baseline (speedup 1.0000x reference)
"""Trainium2 Bass kernel for nn_CrossAttentionLayer (B=32,S=512,D=1024,H=16).

Sharding: pure data-parallel over batch — 4 batches per core on 8 cores.
Per-core dataflow (all matmuls bf16 operands, fp32 PSUM accumulate):
  LN(token-major, DVE bn_stats) -> PE-transpose to feature-major ->
  weight-stationary projections (q^T,k^T feature-major; vv token-major with a
  per-head masked-ones column appended for the softmax denominator) ->
  k-major scores s^T[k,q] (PE) -> exp(s/8) on ACT (mask folded multiplicatively
  into vv + denominator column) -> ctx^T + denominator in one PE matmul ->
  normalize via broadcast-DMA'd reciprocal rows (fused with PSUM->SBUF copy) ->
  Wo projection (feature-major) -> PE-transpose + residual add -> DMA out.

LayerNorm gains/biases are folded host-side into the projection weights
(exact): W' = diag(g) @ W, b' = ln_b @ W + b.
"""

import os
import numpy as np
import ml_dtypes

import concourse.bass as bass
import concourse.tile as tile
from concourse import mybir
from concourse.bass_utils import run_bass_kernel_spmd
from concourse.masks import make_identity

B, S, D, H = 32, 512, 1024, 16
DH = D // H          # 64
P = 128
NCORES = 8
BC = B // NCORES     # 4 batches per core
FC = D // P          # 8 feature chunks
TT = S // P          # 4 token tiles per batch (also key chunks)
EPS = 1e-5
F32 = mybir.dt.float32
BF16 = mybir.dt.bfloat16

_CACHE = {}
last_results = None  # BassKernelResults of the most recent run (for profiling)


def _split_waits(nc, maxw=1):
    """walrus in this container rejects >1 sync-wait per instruction on some
    opcodes; split extra waits onto NoOp instructions inserted just before."""
    n = 0
    for f in nc.m.functions:
        for blk in f.blocks:
            newlist = []
            for inst in blk.instructions:
                si = inst.sync_info
                if si is not None and si.on_wait is not None and len(si.on_wait) > maxw:
                    waits = list(si.on_wait)
                    extra, keep = waits[:-maxw], waits[-maxw:]
                    chunks = [extra[i:i + maxw] for i in range(0, len(extra), maxw)]
                    for ci, ch in enumerate(chunks):
                        nop = mybir.InstNoOp(
                            name=f"{inst.name}-waitsplit-{ci}",
                            engine=inst.engine,
                            ins=[], outs=[],
                            sync_info=mybir.SyncInfo(on_wait=ch, on_update=[]),
                        )
                        newlist.append(nop)
                        n += 1
                    inst.sync_info = mybir.SyncInfo(on_wait=keep, on_update=si.on_update)
                newlist.append(inst)
            blk.instructions[:] = newlist
    return n


def _bcast_part(ap, nparts):
    """Broadcast a [1, ...] AP across nparts partitions for DMA.

    SBUF source APs need a nonzero partition step, so express the repeat as a
    stride-0 *free* dimension instead: [1, nparts, ...] element stream."""
    return bass.AP(tensor=ap.tensor, offset=ap.offset,
                   ap=[[1, 1], [0, nparts]] + list(ap.ap[1:]))


def _build(bv_nonzero, gv_not_one, biases_zero):
    nc = bass.Bass("TRN2", debug=False, num_devices=NCORES)

    qk_d = nc.dram_tensor("qk", [BC * S, D], F32, kind="ExternalInput").ap()
    v_d = nc.dram_tensor("v", [BC * S, D], F32, kind="ExternalInput").ap()
    mb_d = nc.dram_tensor("maskbit", [BC, P, TT], F32, kind="ExternalInput").ap()
    wq_d = nc.dram_tensor("wq", [D, D], BF16, kind="ExternalInput").ap()
    wk_d = nc.dram_tensor("wk", [D, D], BF16, kind="ExternalInput").ap()
    wv_d = nc.dram_tensor("wv", [D, D], BF16, kind="ExternalInput").ap()
    wo_d = nc.dram_tensor("wo", [D, D], BF16, kind="ExternalInput").ap()
    bq_d = nc.dram_tensor("bqc", [P, FC], F32, kind="ExternalInput").ap()
    bk_d = nc.dram_tensor("bkc", [P, FC], F32, kind="ExternalInput").ap()
    bo_d = nc.dram_tensor("boc", [P, FC], F32, kind="ExternalInput").ap()
    bv_d = nc.dram_tensor("bvr", [1, D], BF16, kind="ExternalInput").ap()
    gv_d = nc.dram_tensor("gvr", [1, D], F32, kind="ExternalInput").ap()
    out_d = nc.dram_tensor("out", [BC * S, D], F32, kind="ExternalOutput").ap()

    with tile.TileContext(nc) as tc:
        _body(tc, qk_d, v_d, mb_d, wq_d, wk_d, wv_d, wo_d,
              bq_d, bk_d, bo_d, bv_d, gv_d, out_d, bv_nonzero, gv_not_one,
              biases_zero)

    _split_waits(nc)
    return nc


def _body(tc, qk_d, v_d, mb_d, wq_d, wk_d, wv_d, wo_d,
          bq_d, bk_d, bo_d, bv_d, gv_d, out_d, bv_nonzero, gv_not_one,
          biases_zero):
    nc = tc.nc
    from contextlib import ExitStack
    ctx = ExitStack()
    with ctx:
        singles = ctx.enter_context(tc.tile_pool(name="singles", bufs=1))
        in_pool = ctx.enter_context(tc.tile_pool(name="inp", bufs=2))
        stat_pool = ctx.enter_context(tc.tile_pool(name="stats", bufs=7))
        zr_pool = ctx.enter_context(tc.tile_pool(name="zr", bufs=2))
        zqk_pool = ctx.enter_context(tc.tile_pool(name="zqk", bufs=4))
        zv_pool = ctx.enter_context(tc.tile_pool(name="zv", bufs=5))
        zqkT_pool = ctx.enter_context(tc.tile_pool(name="zqkT", bufs=9))
        zvT_pool = ctx.enter_context(tc.tile_pool(name="zvT", bufs=9))
        qT_pool = ctx.enter_context(tc.tile_pool(name="qT", bufs=9))
        kT_pool = ctx.enter_context(tc.tile_pool(name="kT", bufs=9))
        vv_pool = ctx.enter_context(tc.tile_pool(name="vv", bufs=6))
        exp_pool = ctx.enter_context(tc.tile_pool(name="expp", bufs=4))
        ctxN_pool = ctx.enter_context(tc.tile_pool(name="ctxN", bufs=9))
        stage_pool = ctx.enter_context(tc.tile_pool(name="stg", bufs=2))
        oT_pool = ctx.enter_context(tc.tile_pool(name="oT", bufs=9))
        den_pool = ctx.enter_context(tc.tile_pool(name="den", bufs=2))
        recb_pool = ctx.enter_context(tc.tile_pool(name="recb", bufs=2))
        outs_pool = ctx.enter_context(tc.tile_pool(name="outs", bufs=2))
        mb_pool = ctx.enter_context(tc.tile_pool(name="mb", bufs=2))

        psA = ctx.enter_context(tc.tile_pool(name="psA", bufs=2, space="PSUM"))
        psS = ctx.enter_context(tc.tile_pool(name="psS", bufs=2, space="PSUM"))
        psC = ctx.enter_context(tc.tile_pool(name="psC", bufs=2, space="PSUM"))

        # ---- constants / weights ----
        ident_bf = singles.tile([P, P], BF16)
        make_identity(nc, ident_bf)
        ident_f32 = singles.tile([P, P], F32)
        make_identity(nc, ident_f32)
        eps_t = singles.tile([P, 1], F32)
        nc.vector.memset(eps_t[:], EPS)

        # prefetch batch-0 inputs BEFORE weights so the first LN isn't queued
        # behind 8MB of weight traffic
        pre_in = {}
        mb_sb0 = mb_pool.tile([P, TT], F32, tag="mb_sb")
        nc.sync.dma_start(mb_sb0[:], mb_d[0])
        for t in range(TT):
            qk_t = in_pool.tile([P, D], F32, tag="qkin")
            nc.sync.dma_start(qk_t[:], qk_d[t * P:(t + 1) * P, :])
            v_t = in_pool.tile([P, D], F32, tag="vin")
            nc.sync.dma_start(v_t[:], v_d[t * P:(t + 1) * P, :])
            pre_in[t] = (qk_t, v_t)

        w_sb = {}
        for name, wd in (("wq", wq_d), ("wk", wk_d), ("wv", wv_d), ("wo", wo_d)):
            t = singles.tile([P, FC, D], BF16, tag=name)
            for kc in range(FC):
                nc.sync.dma_start(t[:, kc, :], wd[kc * P:(kc + 1) * P, :])
            w_sb[name] = t
        bq_c = singles.tile([P, FC], F32)
        nc.sync.dma_start(bq_c[:], bq_d)
        bk_c = singles.tile([P, FC], F32)
        nc.sync.dma_start(bk_c[:], bk_d)
        bo_c = singles.tile([P, FC], F32)
        nc.sync.dma_start(bo_c[:], bo_d)
        if bv_nonzero:
            bv_row = singles.tile([1, D], BF16)
            nc.sync.dma_start(bv_row[:], bv_d)
            ones_col = singles.tile([1, P], BF16)
            nc.vector.memset(ones_col[:], 1.0)
        if gv_not_one:
            gv_b = singles.tile([P, D], F32)
            nc.sync.dma_start(gv_b[:], _bcast_part(gv_d, P))

        def emit_LN(b):
            """LayerNorm phase (DVE-heavy): returns (mb_sb, zqk_tiles, zv_tiles)."""
            if b == 0:
                mb_sb = mb_sb0
            else:
                mb_sb = mb_pool.tile([P, TT], F32, tag="mb_sb", name=f"mb_{b}")
                nc.sync.dma_start(mb_sb[:], mb_d[b])
            zqk_tiles, zv_tiles = [], []
            for t in range(TT):
                row0 = (b * TT + t) * P
                if b == 0:
                    qk_t, v_t = pre_in[t]
                else:
                    qk_t = in_pool.tile([P, D], F32, tag="qkin", name=f"qk_{b}_{t}")
                    nc.sync.dma_start(qk_t[:], qk_d[row0:row0 + P, :])
                    v_t = in_pool.tile([P, D], F32, tag="vin", name=f"v_{b}_{t}")
                    nc.sync.dma_start(v_t[:], v_d[row0:row0 + P, :])
                z_pairs = []
                for which, x_t in (("qk", qk_t), ("v", v_t)):
                    st = stat_pool.tile([P, 2, 6], F32, tag="st", name=f"st_{b}_{t}_{which}")
                    nc.vector.bn_stats(st[:, 0, :], x_t[:, 0:512])
                    nc.vector.bn_stats(st[:, 1, :], x_t[:, 512:1024])
                    mv = stat_pool.tile([P, 2], F32, tag="mv", name=f"mv_{b}_{t}_{which}")
                    nc.vector.bn_aggr(mv[:], st[:])
                    std = stat_pool.tile([P, 1], F32, tag="sd", name=f"sd_{b}_{t}_{which}")
                    nc.scalar.activation(std[:], mv[:, 1:2],
                                         mybir.ActivationFunctionType.Sqrt,
                                         bias=eps_t[:])
                    rstd = stat_pool.tile([P, 1], F32, tag="rs", name=f"rs_{b}_{t}_{which}")
                    nc.vector.reciprocal(rstd[:], std[:])
                    z_pairs.append((mv, rstd))
                (mv_q, rs_q), (mv_v, rs_v) = z_pairs
                zqk_t = zqk_pool.tile([P, D], BF16, tag="zqk", name=f"zqk_{b}_{t}")
                nc.vector.tensor_scalar(zqk_t[:], qk_t[:], mv_q[:, 0:1], rs_q[:],
                                        op0=mybir.AluOpType.subtract,
                                        op1=mybir.AluOpType.mult)
                zv_t = zv_pool.tile([P, D], F32, tag="zv", name=f"zv_{b}_{t}")
                nc.vector.tensor_scalar(zv_t[:], v_t[:], mv_v[:, 0:1], rs_v[:],
                                        op0=mybir.AluOpType.subtract,
                                        op1=mybir.AluOpType.mult)
                zqk_tiles.append(zqk_t)
                zv_tiles.append(zv_t)
            return mb_sb, zqk_tiles, zv_tiles

        def emit_TPV(b, state):
            """Transpose + projections (PE-heavy)."""
            mb_sb, zqk_tiles, zv_tiles = state
            zqkT, zvT = [], []
            for fc in range(FC):
                psT = psA.tile([P, 512], BF16, tag="ps", name=f"psT_{b}_{fc}")
                for t in range(TT):
                    nc.tensor.transpose(psT[:, t * P:(t + 1) * P],
                                        zqk_tiles[t][:, fc * P:(fc + 1) * P], ident_bf[:])
                zq = zqkT_pool.tile([P, 512], BF16, tag="zqkT", name=f"zqkT_{b}_{fc}")
                nc.scalar.copy(zq[:], psT[:])
                zqkT.append(zq)
                psT2 = psA.tile([P, 512], F32, tag="ps", name=f"psT2_{b}_{fc}")
                for t in range(TT):
                    nc.tensor.transpose(psT2[:, t * P:(t + 1) * P],
                                        zv_tiles[t][:, fc * P:(fc + 1) * P], ident_f32[:])
                zv = zvT_pool.tile([P, 512], BF16, tag="zvT", name=f"zvT_{b}_{fc}")
                nc.scalar.copy(zv[:], psT2[:])
                zvT.append(zv)

            qT, kT = [], []
            for fc in range(FC):
                psq = psA.tile([P, 512], F32, tag="ps", name=f"psq_{b}_{fc}")
                for kc in range(FC):
                    nc.tensor.matmul(psq[:], w_sb["wq"][:, kc, fc * P:(fc + 1) * P],
                                     zqkT[kc][:], start=(kc == 0), stop=(kc == FC - 1))
                q_t = qT_pool.tile([P, 512], BF16, tag="qT", name=f"qT_{b}_{fc}")
                if biases_zero:
                    nc.scalar.copy(q_t[:], psq[:])
                else:
                    nc.vector.tensor_scalar_add(q_t[:], psq[:], bq_c[:, fc:fc + 1])
                qT.append(q_t)
                psk = psA.tile([P, 512], F32, tag="ps", name=f"psk_{b}_{fc}")
                for kc in range(FC):
                    nc.tensor.matmul(psk[:], w_sb["wk"][:, kc, fc * P:(fc + 1) * P],
                                     zqkT[kc][:], start=(kc == 0), stop=(kc == FC - 1))
                k_t = kT_pool.tile([P, 512], BF16, tag="kT", name=f"kT_{b}_{fc}")
                if biases_zero:
                    nc.scalar.copy(k_t[:], psk[:])
                else:
                    nc.vector.tensor_scalar_add(k_t[:], psk[:], bk_c[:, fc:fc + 1])
                kT.append(k_t)

            vv_ext = []
            for t in range(TT):
                vx = vv_pool.tile([P, H, DH + 1], BF16, tag="vx", name=f"vx_{b}_{t}")
                mb_col = bass.AP(tensor=mb_sb.tensor, offset=mb_sb[:, t:t + 1].offset,
                                 ap=[mb_sb.ap[0], [0, H], [0, 1]])
                nc.vector.tensor_copy(vx[:, :, DH:DH + 1], mb_col)
                for nh in range(2):
                    psv = psA.tile([P, 512], F32, tag="ps", name=f"psv_{b}_{t}_{nh}")
                    first = True
                    if bv_nonzero:
                        nc.tensor.matmul(psv[:], ones_col[:, :P],
                                         bv_row[:, nh * 512:(nh + 1) * 512],
                                         start=True, stop=False)
                        first = False
                    for kc in range(FC):
                        nc.tensor.matmul(psv[:], zvT[kc][:, t * P:(t + 1) * P],
                                         w_sb["wv"][:, kc, nh * 512:(nh + 1) * 512],
                                         start=(first and kc == 0), stop=(kc == FC - 1))
                    nc.vector.tensor_scalar(
                        vx[:, nh * 8:(nh + 1) * 8, 0:DH],
                        psv.rearrange("p (h d) -> p h d", h=8),
                        mb_sb[:, t:t + 1], None, op0=mybir.AluOpType.mult)
                vv_ext.append(vx)
            return qT, kT, vv_ext

        def emit_ATTN(b, proj):
            """Scores -> exp -> ctx+denominator -> normalize (ACT-heavy)."""
            qT, kT, vv_ext = proj
            ctxN = [None] * FC
            for fc in range(FC):
                cx = ctxN_pool.tile([P, 512], BF16, tag="cx", name=f"cx_{b}_{fc}")
                ctxN[fc] = cx
                exp_hs = [exp_pool.tile([P, TT, 512], BF16, tag="exp",
                                        name=f"exp_{b}_{fc}_{i}") for i in range(2)]
                for g in range(2):
                    sgs = [psS.tile([P, 2, 512], F32, tag="sg",
                                    name=f"sg_{b}_{fc}_{g}_{i}") for i in range(2)]
                    for j in range(2):
                        kc = g * 2 + j
                        for half in range(2):
                            p0 = half * DH
                            nc.tensor.matmul(sgs[half][:, j, :],
                                             kT[fc][p0:p0 + DH, kc * P:(kc + 1) * P],
                                             qT[fc][p0:p0 + DH, :],
                                             start=True, stop=True)
                    for half in range(2):
                        nc.scalar.activation(exp_hs[half][:, g * 2:(g + 1) * 2, :],
                                             sgs[half][:],
                                             mybir.ActivationFunctionType.Exp,
                                             scale=0.125)
                for half in range(2):
                    h = fc * 2 + half
                    exp_h = exp_hs[half]
                    psc = psC.tile([DH + 1, 512], F32, tag="psc", name=f"psc_{b}_{fc}_{half}")
                    for kc in range(TT):
                        nc.tensor.matmul(psc[:], vv_ext[kc][:, h, :], exp_h[:, kc, :],
                                         start=(kc == 0), stop=(kc == TT - 1))
                    rec = den_pool.tile([1, 512], F32, tag="rec", name=f"rec_{b}_{fc}_{half}")
                    nc.vector.reciprocal(rec[:], psc[DH:DH + 1, :])
                    recb = recb_pool.tile([DH, 512], F32, tag="recb", name=f"recb_{b}_{fc}_{half}")
                    nc.sync.dma_start(recb[:], _bcast_part(rec, DH))
                    if half == 0:
                        nc.vector.tensor_mul(cx[0:DH, :], psc[0:DH, :], recb[:])
                    else:
                        stg = stage_pool.tile([DH, 512], BF16, tag="stg", name=f"stg_{b}_{fc}")
                        nc.vector.tensor_mul(stg[:], psc[0:DH, :], recb[:])
                        nc.sync.dma_start(cx[DH:P, :], stg[:])
            return ctxN

        def emit_O_ASM(b, ctxN, zv_tiles):
            oT = []
            for fc in range(FC):
                pso = psA.tile([P, 512], F32, tag="ps", name=f"pso_{b}_{fc}")
                for kc in range(FC):
                    nc.tensor.matmul(pso[:], w_sb["wo"][:, kc, fc * P:(fc + 1) * P],
                                     ctxN[kc][:], start=(kc == 0), stop=(kc == FC - 1))
                o_t = oT_pool.tile([P, 512], BF16, tag="oT", name=f"oT_{b}_{fc}")
                nc.vector.tensor_scalar_add(o_t[:], pso[:], bo_c[:, fc:fc + 1])
                oT.append(o_t)
            zv_tiles = all_zv[b]
            for t in range(TT):
                row0 = (b * TT + t) * P
                if gv_not_one:
                    zr = zr_pool.tile([P, D], F32, tag="zr", name=f"zr_{b}_{t}")
                    nc.gpsimd.tensor_mul(zr[:], zv_tiles[t][:], gv_b[:])
                else:
                    zr = zv_tiles[t]
                for half in range(2):
                    pa = psA.tile([P, 512], BF16, tag="ps", name=f"pa_{b}_{t}_{half}")
                    for j in range(4):
                        fc = half * 4 + j
                        nc.tensor.transpose(pa[:, j * P:(j + 1) * P],
                                            oT[fc][:, t * P:(t + 1) * P], ident_bf[:])
                    osb = outs_pool.tile([P, 512], F32, tag="osb", name=f"osb_{b}_{t}_{half}")
                    nc.vector.tensor_add(osb[:], pa[:], zr[:, half * 512:(half + 1) * 512])
                    nc.sync.dma_start(out_d[row0:row0 + P, half * 512:(half + 1) * 512],
                                      osb[:])

        all_zv = {}

        # software pipeline: LN(b+1) is emitted before ATTN(b) so its DVE work
        # fills the ACT-bound attention phase instead of queueing behind the
        # whole batch on the in-order engines.
        ln_state = emit_LN(0)
        for b in range(BC):
            all_zv[b] = ln_state[2]
            proj = emit_TPV(b, ln_state)
            if b + 1 < BC:
                ln_next = emit_LN(b + 1)
            ctxN = emit_ATTN(b, proj)
            emit_O_ASM(b, ctxN, None)
            if b + 1 < BC:
                ln_state = ln_next

def _prep_host(inputs):
    """Host-side exact folding of LN gains/biases into projection weights."""
    f32 = np.float32
    qk = np.asarray(inputs["qk"], f32)
    v = np.asarray(inputs["v"], f32)
    mask = np.asarray(inputs["mask"])
    g_qk = np.asarray(inputs["qk_ln_g"], f32)
    b_qk = np.asarray(inputs["qk_ln_b"], f32)
    g_v = np.asarray(inputs["v_ln_g"], f32)
    b_v = np.asarray(inputs["v_ln_b"], f32)
    Wq = np.asarray(inputs["Wq"], f32)
    Wk = np.asarray(inputs["Wk"], f32)
    Wv = np.asarray(inputs["Wv"], f32)
    Wo = np.asarray(inputs["Wo"], f32)
    bq = np.asarray(inputs["bq"], f32)
    bk = np.asarray(inputs["bk"], f32)
    bv = np.asarray(inputs["bv"], f32)
    bo = np.asarray(inputs["bo"], f32)

    wq_f = (g_qk[:, None] * Wq).astype(ml_dtypes.bfloat16)
    wk_f = (g_qk[:, None] * Wk).astype(ml_dtypes.bfloat16)
    wv_f = (g_v[:, None] * Wv).astype(ml_dtypes.bfloat16)
    wo_f = Wo.astype(ml_dtypes.bfloat16)
    bq_f = (b_qk @ Wq + bq).astype(f32)
    bk_f = (b_qk @ Wk + bk).astype(f32)
    bv_f = (b_v @ Wv + bv).astype(f32)
    # residual constant (b_v from v LN) is folded into the output bias
    bo_f = (bo + b_v).astype(f32)

    bv_nonzero = bool(np.any(bv_f != 0))
    gv_not_one = not bool(np.all(g_v == 1.0))
    biases_zero = not (np.any(bq_f) or np.any(bk_f) or np.any(bo_f))

    maskbit = mask.astype(f32).reshape(B, TT, P).transpose(0, 2, 1).copy()  # [B,P,TT]

    per_core = []
    for c in range(NCORES):
        sl = slice(c * BC, (c + 1) * BC)
        im = {
            "qk": np.ascontiguousarray(qk[sl].reshape(BC * S, D)),
            "v": np.ascontiguousarray(v[sl].reshape(BC * S, D)),
            "maskbit": np.ascontiguousarray(maskbit[sl]),
            "wq": wq_f, "wk": wk_f, "wv": wv_f, "wo": wo_f,
            "bqc": np.ascontiguousarray(bq_f.reshape(FC, P).T),
            "bkc": np.ascontiguousarray(bk_f.reshape(FC, P).T),
            "boc": np.ascontiguousarray(bo_f.reshape(FC, P).T),
            "bvr": bv_f.reshape(1, D).astype(ml_dtypes.bfloat16),
            "gvr": g_v.reshape(1, D).astype(f32),
        }
        per_core.append(im)
    return per_core, bv_nonzero, gv_not_one, biases_zero


def kernel(**inputs):
    global last_results
    per_core, bv_nonzero, gv_not_one, biases_zero = _prep_host(inputs)
    key = (bv_nonzero, gv_not_one, biases_zero)
    if key not in _CACHE:
        _CACHE[key] = _build(bv_nonzero, gv_not_one, biases_zero)
    nc = _CACHE[key]
    res = run_bass_kernel_spmd(nc, per_core, core_ids=list(range(NCORES)))
    last_results = res
    out = np.concatenate([res.results[c]["out"].reshape(BC, S, D)
                          for c in range(NCORES)], axis=0)
    return out.astype(np.float32)

